# revision 1
# baseline (speedup 1.0000x reference)
"""Trainium2 Bass kernel for nn_AutoregressivePPRM.

Model (per sequence row): 24-step autoregressive GRU (input_size=1, hidden=256)
whose scalar input at each step is the previous step's prediction
pred = (h @ Wl.T + bl) @ Wh.T + bh  -- affine in h. We exploit that to fold the
input path into the recurrent weights:

  x_{t+1} = w_e @ h_t + b_e            (w_e = Wh@Wl [1,256], b_e scalar)
  gi_rz   = W_ih_rz * x + b_ih_rz  =>  folded: W_rz_eff = W_hh_rz + W_ih_rz @ w_e
  inn     = w_ih_n * x_t + b_ih_n      (rank-1: K=1 matmul from the pred row)

Layout: everything transposed on-chip: hidden dim on partitions, B*N rows on
the free axis. Host pre-transposes features and post-transposes the output.
Data parallel: 16000 rows sharded 2000/core across 8 cores; weights replicated.
"""

import os

import numpy as np

import concourse.bass as bass
import concourse.tile as tile
from concourse import bacc
from concourse import mybir
from concourse.bass_utils import run_bass_kernel_spmd

B, N, D, HOR, BOT = 32, 500, 256, 24, 8
NCORES = 8
ROWS = B * N // NCORES          # 2000 rows per core
RT = 4                          # row tiles per core
W = ROWS // RT                  # 500 columns per row tile
KT = D // 128                   # 2 contraction tiles

F32 = mybir.dt.float32
MDT = mybir.dt.float32r         # matmul compute dtype (full-rate fp32 mode)






PK_OFF = {}
_c = 0
for _n, _w in [("wg1", 16), ("w_rz1", 1024), ("w_hn", 512), ("wirz", 512),
               ("winb", 256), ("w_e", 2), ("one", W), ("lv", ROWS),
               ("w_rz", 1024), ("wdk", HOR), ("wg2", HOR)]:
    PK_OFF[_n] = _c
    _c += _w
PK_TOT = _c
PK_CUT = PK_OFF["winb"]  # first DMA chunk: step-1-critical weights
PK_CUT2 = PK_OFF["w_rz"]

BP_OFF = {}
_c = 0
for _n, _w in [("brz", 4), ("brz1", 4), ("bhn", KT), ("bin", KT),
               ("bg1", 1), ("bg2", 1), ("be", 1)]:
    BP_OFF[_n] = _c
    _c += _w
BP_TOT = _c


def build_nc():
    nc = bacc.Bacc()

    dram = {}

    def param(name, shape, out=False, dt=None):
        dram[name] = nc.declare_dram_parameter(
            name, list(shape), dt or F32, isOutput=out
        )
        return dram[name]

    hT_d = param("hT", (KT, 128, ROWS), dt=MDT)
    wpk_d = param("wpk", (128, PK_TOT), dt=MDT)   # matmul weights, packed
    bpk_d = param("bpk", (128, BP_TOT))           # f32 biases, packed
    out_d = param("out", (HOR, ROWS), out=True)

    with tile.TileContext(nc, trace_sim=bool(os.environ.get('KTRACE')), pool_alloc_mode='stack') as tc:
        with (
            tc.tile_pool(name="wp", bufs=1) as wp,
            tc.tile_pool(name="hp", bufs=1) as hp,
            tc.tile_pool(name="gp", bufs=5) as gp,
            tc.tile_pool(name="pp", bufs=1, space="PSUM") as pp,
        ):
            # ---- load weights/biases (single packed DMA) ----
            wpk = wp.tile([128, PK_TOT], MDT, tag="wpk")
            nc.sync.dma_start(out=wpk[:, 0:PK_CUT], in_=wpk_d[:, 0:PK_CUT])
            nc.scalar.dma_start(out=wpk[:, PK_CUT:PK_CUT2],
                                in_=wpk_d[:, PK_CUT:PK_CUT2])
            nc.scalar.dma_start(out=wpk[:, PK_CUT2:], in_=wpk_d[:, PK_CUT2:])
            O = PK_OFF
            w_rz = wpk[:, O["w_rz"]:O["w_rz"] + KT * 512].rearrange(
                "p (k n) -> p k n", k=KT)
            w_rz1 = wpk[:, O["w_rz1"]:O["w_rz1"] + KT * 512].rearrange(
                "p (k n) -> p k n", k=KT)
            w_hn = wpk[:, O["w_hn"]:O["w_hn"] + KT * 256].rearrange(
                "p (k n) -> p k n", k=KT)
            w_e = wpk[:, O["w_e"]:O["w_e"] + KT].rearrange(
                "p (k n) -> p k n", k=KT)
            wg1 = wpk[:, O["wg1"]:O["wg1"] + KT * BOT].rearrange(
                "p (k n) -> p k n", k=KT)
            bpk = wp.tile([128, BP_TOT], F32, tag="bpk")
            nc.sync.dma_start(out=bpk, in_=bpk_d[:])
            OB = BP_OFF
            brz = bpk[:, OB["brz"]:OB["brz"] + 4]
            brz1 = bpk[:, OB["brz1"]:OB["brz1"] + 4]
            bhn = bpk[:, OB["bhn"]:OB["bhn"] + KT]
            bin_ = bpk[:, OB["bin"]:OB["bin"] + KT]
            wirz = wpk[0:1, O["wirz"]:O["wirz"] + 512]
            winb = wpk[0:2, O["winb"]:O["winb"] + 256]
            wdk = wpk[0:1, O["wdk"]:O["wdk"] + HOR]
            wg2 = wpk[0:BOT, O["wg2"]:O["wg2"] + HOR]
            bg1 = bpk[0:BOT, OB["bg1"]:OB["bg1"] + 1]
            bg2 = bpk[0:HOR, OB["bg2"]:OB["bg2"] + 1]
            be = bpk[0:1, OB["be"]:OB["be"] + 1]
            lv = wpk[0:1, O["lv"]:O["lv"] + ROWS]

            # ---- hidden state (= features, transposed), per row tile ----
            h = []
            for r in range(RT):
                h_r = hp.tile([128, KT, W], MDT, tag=f"h{r}")
                for k in range(KT):
                    nc.gpsimd.dma_start(
                        out=h_r[:, k, :], in_=hT_d[k, :, r * W:(r + 1) * W]
                    )
                h.append(h_r)

            preds = [hp.tile([HOR, W], MDT, tag=f"preds{r}", name=f"preds{r}")
                     for r in range(RT)]
            x2 = [hp.tile([2, W], MDT, tag=f"x2_{r}", name=f"x2_{r}")
                  for r in range(RT)]
            one_row = wpk[0:1, O["one"]:O["one"] + W]
            for r in range(RT):
                nc.sync.dma_start(out=x2[r][1:2, :], in_=one_row)
                nc.sync.dma_start(out=x2[r][0:1, :],
                                  in_=lv[0:1, r * W:(r + 1) * W])
            gate = hp.tile([HOR, ROWS], F32, tag="gate")
            g1 = hp.tile([BOT, ROWS], MDT, tag="g1")

            SIG = mybir.ActivationFunctionType.Sigmoid
            TANH = mybir.ActivationFunctionType.Tanh
            RELU = mybir.ActivationFunctionType.Relu
            IDENT = mybir.ActivationFunctionType.Identity
            ADD = mybir.AluOpType.add
            MULT = mybir.AluOpType.mult

            # ---- prologue: mixing gate from h0 (also warms up the PE) ----
            for r in range(RT):
                cols = slice(r * W, (r + 1) * W)
                pg1 = pp.tile([BOT, W], F32, tag="hn0")
                for k in range(KT):
                    nc.tensor.matmul(
                        pg1[:], (wg1[:, k, :]), (h[r][:, k, :]),
                        start=(k == 0), stop=(k == KT - 1),
                    )
                nc.scalar.activation(g1[:, cols], pg1[:], RELU, bias=bg1[:, 0:1])
                pg2 = pp.tile([HOR, W], F32, tag="hn1")
                nc.tensor.matmul(pg2[:], (wg2[:]), (g1[:, cols]),
                                 start=True, stop=True)
                nc.scalar.activation(gate[:, cols], pg2[:], SIG, bias=bg2[:, 0:1])

            # ---- 24 GRU steps ----
            for u in range(1, HOR + 1):
                first = u == 1
                w_cur = w_rz1 if first else w_rz
                b_cur = brz1 if first else brz
                for r in range(RT):
                    cols = slice(r * W, (r + 1) * W)
                    # pred_{u-1} = w_e @ h_{u-1} + b_e  (becomes x_u)
                    x_row = x2[r][0:1, :]
                    if not first:
                        px = pp.tile([1, W], F32, tag="rz0")
                        for k in range(KT):
                            nc.tensor.matmul(
                                px[:], (w_e[:, k, :]), (h[r][:, k, :]),
                                start=(k == 0), stop=(k == KT - 1),
                            )
                        nc.vector.tensor_scalar_add(x_row, px[:],
                                                    be[0:1, 0:1])
                        nc.sync.dma_start(out=preds[r][u - 2:u - 1, :],
                                          in_=x_row)

                    # G matmuls: rz (4 tiles, folded), hn (2), inn (2, K=1)
                    prz = []
                    for m in (1, 2, 3, 0):
                        p = pp.tile([128, W], F32, tag=f"rz{m}")
                        for k in range(KT):
                            nc.tensor.matmul(
                                p[:], (w_cur[:, k, m * 128:(m + 1) * 128]),
                                (h[r][:, k, :]),
                                start=(k == 0), stop=(k == KT - 1 and not first),
                            )
                        if first:
                            nc.tensor.matmul(
                                p[:], (wirz[0:1, m * 128:(m + 1) * 128]),
                                (x_row), start=False, stop=True,
                            )
                        prz.append((m, p))
                    phn = []
                    for m in range(KT):
                        p = pp.tile([128, W], F32, tag=f"hn{m}")
                        for k in range(KT):
                            nc.tensor.matmul(
                                p[:], (w_hn[:, k, m * 128:(m + 1) * 128]),
                                (h[r][:, k, :]),
                                start=(k == 0), stop=(k == KT - 1),
                            )
                        phn.append(p)
                    pinn = pp.tile([128, KT, 512], F32, tag="inn")
                    for m in range(KT):
                        # K=2: [w_ih_n; b_ih_n] @ [x; 1] = w*x + b
                        nc.tensor.matmul(
                            pinn[:, m, 0:W], (winb[0:2, m * 128:(m + 1) * 128]),
                            (x2[r][0:2, :]), start=True, stop=True,
                        )

                    # gates
                    r_sb = gp.tile([128, KT, W], F32, tag="r")
                    z_sb = gp.tile([128, KT, W], F32, tag="z")
                    for m, p in prz:
                        if m < KT:
                            nc.scalar.activation(
                                r_sb[:, m, :], p[:], SIG, bias=b_cur[:, m:m + 1]
                            )
                        else:
                            nc.scalar.activation(
                                z_sb[:, m - KT, :], p[:], SIG,
                                bias=b_cur[:, m:m + 1],
                            )
                    ncand = gp.tile([128, KT, W], F32, tag="ncand")
                    t_sb = gp.tile([128, KT, W], F32, tag="t")
                    for m in range(KT):
                        # (hn + b_hh_n) * r
                        nc.vector.scalar_tensor_tensor(
                            t_sb[:, m, :], phn[m][:], bhn[:, m:m + 1],
                            r_sb[:, m, :], op0=ADD, op1=MULT,
                        )
                    # + inn (both halves in one op)
                    nc.vector.tensor_add(t_sb[:, :, :], t_sb[:, :, :],
                                         pinn[:, :, 0:W])
                    nc.scalar.activation(ncand[:, :, :], t_sb[:, :, :],
                                         TANH, bias=0.0)
                    # h = ncand + z * (h - ncand)
                    d_sb = gp.tile([128, KT, W], F32, tag="d")
                    nc.gpsimd.tensor_sub(d_sb[:], h[r][:, :, :], ncand[:])
                    nc.gpsimd.tensor_mul(d_sb[:], d_sb[:], z_sb[:])
                    nc.gpsimd.tensor_add(h[r][:, :, :], ncand[:], d_sb[:])

            # ---- final pred_24 ----
            for r in range(RT):
                px = pp.tile([1, W], F32, tag="rz0")
                for k in range(KT):
                    nc.tensor.matmul(
                        px[:], (w_e[:, k, :]), (h[r][:, k, :]),
                        start=(k == 0), stop=(k == KT - 1),
                    )
                nc.vector.tensor_scalar_add(x2[r][0:1, :], px[:],
                                             be[0:1, 0:1])
                nc.sync.dma_start(out=preds[r][HOR - 1:HOR, :],
                                  in_=x2[r][0:1, :])

            # ---- epilogue: out = decay + gate * (preds - decay) ----
            for r in range(RT):
                cols = slice(r * W, (r + 1) * W)
                pdec = pp.tile([HOR, W], F32, tag="hn0")
                nc.tensor.matmul(pdec[:], (wdk[:]), (lv[0:1, cols]),
                                 start=True, stop=True)
                td = gp.tile([HOR, W], F32, tag="td")
                nc.vector.tensor_sub(td[:], preds[r][:], pdec[:])
                nc.gpsimd.tensor_mul(td[:], td[:], gate[:, cols])
                out_sb = gp.tile([HOR, W], F32, tag="osb")
                nc.vector.tensor_add(out_sb[:], td[:], pdec[:])
                nc.sync.dma_start(out=out_d[:, cols], in_=out_sb[:])

    nc.finalize()
    return nc


_NC_CACHE = None


def _get_nc():
    global _NC_CACHE
    if _NC_CACHE is None:
        _NC_CACHE = build_nc()
    return _NC_CACHE


def kernel(features, last_value, W_ih, W_hh, b_ih, b_hh, Wl, bl, Wh, bh,
           Wg1, bg1, Wg2, bg2, log_decay):
    features = np.asarray(features, np.float32)
    last_value = np.asarray(last_value, np.float32)
    f64 = lambda a: np.asarray(a, np.float64)
    W_ih, W_hh, b_ih, b_hh = map(f64, (W_ih, W_hh, b_ih, b_hh))
    Wl, bl, Wh, bh = map(f64, (Wl, bl, Wh, bh))
    Wg1, bg1, Wg2, bg2 = map(f64, (Wg1, bg1, Wg2, bg2))

    w_e = (Wh @ Wl)[0]                      # [256]
    b_e = float((Wh @ bl + bh)[0])
    W_rz_eff = W_hh[0:512] + W_ih[0:512] @ w_e[None, :]
    b_rz_eff = b_hh[0:512] + b_ih[0:512] + W_ih[0:512, 0] * b_e
    b_rz1 = b_hh[0:512] + b_ih[0:512]
    t = np.arange(1, HOR + 1, dtype=np.float64)
    decay_curve = np.exp(-np.exp(float(log_decay)) * t)

    def pack_kpn(arr_t):  # [D, M] -> [128, KT*M] laid out (p, k*M+n)
        kt = arr_t.reshape(KT, 128, -1)
        return np.transpose(kt, (1, 0, 2)).reshape(128, -1)

    NPDT = mybir.dt.np(MDT)
    pk = np.zeros((128, PK_TOT), NPDT)
    bp = np.zeros((128, BP_TOT), np.float32)
    O = PK_OFF
    OB = BP_OFF

    def put(name, block):
        block = np.asarray(block, NPDT)
        pk[:block.shape[0], O[name]:O[name] + block.shape[1]] = block

    def putb(name, block):
        block = np.asarray(block, np.float32)
        bp[:block.shape[0], OB[name]:OB[name] + block.shape[1]] = block

    put("w_rz", pack_kpn(W_rz_eff.T))
    put("w_rz1", pack_kpn(W_hh[0:512].T))
    put("w_hn", pack_kpn(W_hh[512:768].T))
    put("w_e", pack_kpn(w_e[:, None]))
    put("wg1", pack_kpn(Wg1.T))
    putb("brz", b_rz_eff.reshape(4, 128).T)
    putb("brz1", b_rz1.reshape(4, 128).T)
    putb("bhn", b_hh[512:768].reshape(KT, 128).T)

    put("wirz", W_ih[0:512, 0][None, :])
    put("winb", np.stack([W_ih[512:768, 0], b_ih[512:768]]))
    put("wdk", decay_curve[None, :])
    put("wg2", Wg2.T)
    put("one", np.ones((1, W), np.float64))
    putb("bg1", bg1[:, None])
    putb("bg2", bg2[:, None])
    putb("be", np.full((1, 1), b_e, np.float32))

    feat_flat = features.reshape(B * N, D)
    lv_flat = last_value.reshape(B * N)
    in_maps = []
    for i in range(NCORES):
        rows = slice(i * ROWS, (i + 1) * ROWS)
        pk_i = pk.copy()
        pk_i[0, O["lv"]:O["lv"] + ROWS] = lv_flat[rows].astype(NPDT)
        m = {
            "hT": np.ascontiguousarray(
                feat_flat[rows].T.reshape(KT, 128, ROWS)).astype(NPDT),
            "wpk": pk_i,
            "bpk": bp,
        }
        in_maps.append(m)

    nc = _get_nc()
    try:
        res = run_bass_kernel_spmd(nc, in_maps, core_ids=list(range(NCORES)))
    except Exception:
        res = run_bass_kernel_spmd(nc, in_maps, core_ids=list(range(NCORES)))
    global LAST_RESULT
    LAST_RESULT = res
    out = np.concatenate([r["out"].T for r in res.results], axis=0)
    return np.ascontiguousarray(out.reshape(B, N, HOR), np.float32)


LAST_RESULT = None

